# revision 1
# baseline (speedup 1.0000x reference)
"""Trainium2 Bass kernel for nn_MemoryAdapterLayer (8-core SPMD).

reference:
    query = x @ Wq.T + bq                  # [B,S,DM]
    scores = query @ memory.T              # [B,S,M] (per batch)
    weights = softmax(scores, -1)
    attended = weights @ memory            # [B,S,DM]
    transformed = attended @ Wm.T + bm     # [B,S,DQ]
    return (x, transformed)

Sharding: 8 cores = (batch b = core//2) x (sequence half h = core%2).
Each core computes transformed for its [1024, :] slice of one batch.
x is passed through on the host.

On-chip layout is fully transposed ("T" = feature-on-partition):
  step1  QT[d,s]   = sum_q WqT[q,d] * xT[q,s]          (f32r)
  step2  sT[m,s]   = sum_d memT[d,m] * QT[d,s]         (f32r)
  exp    eT[m,s]   = exp(sT - 64)                      (ACT, fused shift)
  step4  aT[d,s]   = sum_m memA[m,d] * eT[m,s]         (f32r)
         Z[s]      = sum_m eT[m,s]                     (ones-matmul)
  norm   attT      = aT * (1/Z) broadcast              (DVE; fp16 out)
  step5  tT[q,s]   = sum_d WmT[d,q] * attT[d,s] + bm   (fp16)

float32r runs the PE at full rate (1 cyc/row at N=512) with ~1.5e-4
per-product precision, which keeps the softmax-amplified score error
small; the fixed -64 shift is safe for this problem's score range
(row max in [50, 90], fp32 exp handles s-64 in [-200, 60]).

All DMAs go through SWDGE (gpsimd): this container's walrus rejects
HWDGE semaphore waits on PE instructions. split_overflow_waits() caps
per-instruction sync waits at 1 (S3_LW/CTRL_NO slot limits here).
"""
import sys

import numpy as np

for _p in ("/opt/trn_rl_repo",):
    if _p not in sys.path:
        sys.path.insert(0, _p)

import concourse.bass as bass
import concourse.mybir as mybir
from concourse import tile
from concourse.bass_utils import run_bass_kernel_spmd

B, S, M = 4, 2048, 4096
DQ, DM = 1024, 512
N_CORES = 8
SL = S // 2          # per-core sequence rows
NBLK = 2             # s-blocks of 512 per core
SB = 512             # s-block width (fp32 moving-operand max)
QT_T, DT_T, MT_T = DQ // 128, DM // 128, M // 128  # 8, 4, 32
SHIFT = 80.0

F32R = mybir.dt.float32r
F32 = mybir.dt.float32
F16 = mybir.dt.float16

_counter = [0]


def _split_overflow_waits(nc, limit=1):
    """Walrus here rejects >1 sync wait per instruction: hoist excess waits
    onto same-engine NOPs inserted directly before the instruction."""
    for bb in nc.main_func.blocks:
        insts = list(bb.instructions)
        out = []
        dirty = False
        for ins in insts:
            si = ins.sync_info
            waits = list(si.on_wait) if si is not None else []
            if len(waits) > limit:
                extra = waits[: len(waits) - limit]
                keep = waits[len(waits) - limit:]
                for w in extra:
                    _counter[0] += 1
                    nop = mybir.InstNoOp(
                        name=f"waitfix-{_counter[0]}",
                        engine=ins.engine,
                        sync_info=mybir.SyncInfo(on_wait=[w], on_update=[]),
                        bass_nofuse=True,
                    )
                    nc.register_instruction(nop, overwrite=True)
                    out.append(nop)
                ins.sync_info = mybir.SyncInfo(
                    on_wait=keep, on_update=list(si.on_update)
                )
                dirty = True
            out.append(ins)
        if dirty:
            bb.instructions = out


def build(repeats=1):
    from contextlib import ExitStack

    nc = bass.Bass("TRN2", debug=False, num_devices=N_CORES)
    AF = mybir.ActivationFunctionType

    xT_d = nc.dram_tensor("xT", [128, NBLK * QT_T * SB], F32R, kind="ExternalInput").ap()
    wqT_d = nc.dram_tensor("wqT", [128, QT_T * DT_T * 128], F32R, kind="ExternalInput").ap()
    memT_d = nc.dram_tensor("memT", [128, MT_T * 512], F32R, kind="ExternalInput").ap()
    memA_d = nc.dram_tensor("memA", [128, MT_T * 512], F32R, kind="ExternalInput").ap()
    wmT_d = nc.dram_tensor("wmT", [128, DT_T * QT_T * 128], F16, kind="ExternalInput").ap()
    bqT_d = nc.dram_tensor("bqT", [128, DT_T], F32, kind="ExternalInput").ap()
    bmT_d = nc.dram_tensor("bmT", [128, QT_T], F32, kind="ExternalInput").ap()
    outT_d = nc.dram_tensor("outT", [128, NBLK * QT_T * SB], F32, kind="ExternalOutput").ap()

    with tile.TileContext(nc) as tc:
        with ExitStack() as ctx:
            res = ctx.enter_context(tc.tile_pool(name="res", bufs=1))
            qtp = ctx.enter_context(tc.tile_pool(name="qtp", bufs=8))
            mtp = ctx.enter_context(tc.tile_pool(name="mtp", bufs=6))
            exp = ctx.enter_context(tc.tile_pool(name="expp", bufs=4))
            att = ctx.enter_context(tc.tile_pool(name="attp", bufs=8))
            bcp = ctx.enter_context(tc.tile_pool(name="bcp", bufs=2))
            otp = ctx.enter_context(tc.tile_pool(name="otp", bufs=4))
            ps = ctx.enter_context(tc.tile_pool(name="ps", bufs=3, space="PSUM"))
            psa = ctx.enter_context(tc.tile_pool(name="psa", bufs=1, space="PSUM"))

            # resident tensors
            xT = res.tile([128, NBLK * QT_T * SB], F32R)
            wqT = res.tile([128, QT_T * DT_T * 128], F32R)
            memA = res.tile([128, MT_T * 512], F32R)
            wmT = res.tile([128, DT_T * QT_T * 128], F16)
            bqT = res.tile([128, DT_T], F32)
            bmT = res.tile([128, QT_T], F32)
            ones = res.tile([128, 1], F32)
            onesr = res.tile([1, 128], F32)
            neg64 = res.tile([128, 1], F32)
            nc.gpsimd.dma_start(xT[:], xT_d)
            nc.gpsimd.dma_start(wqT[:], wqT_d)
            nc.gpsimd.dma_start(memA[:], memA_d)
            nc.gpsimd.dma_start(wmT[:], wmT_d)
            nc.gpsimd.dma_start(bqT[:], bqT_d)
            nc.gpsimd.dma_start(bmT[:], bmT_d)
            nc.gpsimd.memset(ones[:], 1.0)
            nc.gpsimd.memset(onesr[:], 1.0)
            nc.gpsimd.memset(neg64[:], -SHIFT)

            for _rep in range(repeats):
                for blk in range(NBLK):
                    # ---- step1: QT[dt] = WqT.T @ xT + bq ----
                    QT = []
                    for dt in range(DT_T):
                        pq = ps.tile([128, SB], F32, tag="mm")
                        for qt in range(QT_T):
                            nc.tensor.matmul(
                                pq[:],
                                wqT[:, (qt * DT_T + dt) * 128:(qt * DT_T + dt + 1) * 128],
                                xT[:, (blk * QT_T + qt) * SB:(blk * QT_T + qt + 1) * SB],
                                start=(qt == 0), stop=(qt == QT_T - 1),
                            )
                        q_t = qtp.tile([128, SB], F32R, tag="qt")
                        nc.scalar.activation(q_t[:], pq[:], AF.Identity,
                                             bias=bqT[:, dt:dt + 1])
                        QT.append(q_t)

                    # ---- steps 2-4 fused over memory tiles ----
                    accs = [psa.tile([128, SB], F32, tag=f"acc{i}", name=f"acc{i}") for i in range(DT_T)]
                    sums = psa.tile([1, SB], F32, tag="sums")
                    zacc = bcp.tile([128, SB], F32, tag="zacc")
                    for j in range(MT_T):
                        mt = mtp.tile([128, 512], F32R, tag="mt")
                        nc.gpsimd.dma_start(mt[:], memT_d[:, j * 512:(j + 1) * 512])
                        ss = ps.tile([128, SB], F32, tag="mm")
                        for dt in range(DT_T):
                            nc.tensor.matmul(
                                ss[:], mt[:, dt * 128:(dt + 1) * 128], QT[dt][:],
                                start=(dt == 0), stop=(dt == DT_T - 1),
                            )
                        ex = exp.tile([128, SB], F32R, tag="ex")
                        nc.scalar.activation(ex[:], ss[:], AF.Exp, bias=neg64[:])
                        for dt in range(DT_T):
                            nc.tensor.matmul(
                                accs[dt][:],
                                memA[:, j * 512 + dt * 128: j * 512 + (dt + 1) * 128],
                                ex[:],
                                start=(j == 0), stop=(j == MT_T - 1),
                            )
                        exf = ex[:].bitcast(F32)
                        if j == 0:
                            nc.vector.tensor_copy(zacc[:], exf)
                        else:
                            nc.vector.tensor_add(zacc[:], zacc[:], exf)

                    # ---- normalize: attT = accs * (1/Z) ----
                    nc.tensor.matmul(sums[:], ones[:], zacc[:],
                                     start=True, stop=True)
                    rc = bcp.tile([1, SB], F32, tag="rc")
                    nc.vector.reciprocal(rc[:], sums[:])
                    bc_ps = ps.tile([128, SB], F32, tag="mm")
                    nc.tensor.matmul(bc_ps[:], onesr[:], rc[:], start=True, stop=True)
                    bc = bcp.tile([128, SB], F32, tag="bc")
                    nc.scalar.activation(bc[:], bc_ps[:], AF.Copy)
                    ATT = []
                    for dt in range(DT_T):
                        a_t = att.tile([128, SB], F16, tag="att")
                        nc.vector.tensor_mul(a_t[:], accs[dt][:], bc[:])
                        ATT.append(a_t)

                    # ---- step5: out[qt] = WmT.T @ attT + bm ----
                    for qt in range(QT_T):
                        p5 = ps.tile([128, SB], F32, tag="mm")
                        for dt in range(DT_T):
                            nc.tensor.matmul(
                                p5[:],
                                wmT[:, (dt * QT_T + qt) * 128:(dt * QT_T + qt + 1) * 128],
                                ATT[dt][:],
                                start=(dt == 0), stop=(dt == DT_T - 1),
                            )
                        ot = otp.tile([128, SB], F32, tag="ot")
                        nc.scalar.activation(ot[:], p5[:], AF.Identity,
                                             bias=bmT[:, qt:qt + 1])
                        nc.gpsimd.dma_start(
                            outT_d[:, (blk * QT_T + qt) * SB:(blk * QT_T + qt + 1) * SB],
                            ot[:],
                        )
    _split_overflow_waits(nc)
    return nc


def pack_inputs(x, memory, Wq, bq, Wm, bm):
    """Host-side pre-swizzle into SBUF-shaped [128, F] per-core arrays."""
    f32 = np.float32
    wqT = np.ascontiguousarray(
        Wq.reshape(DT_T, 128, QT_T, 128).transpose(3, 2, 0, 1).reshape(128, -1), f32)
    wmT = np.ascontiguousarray(
        Wm.reshape(QT_T, 128, DT_T, 128).transpose(3, 2, 0, 1).reshape(128, -1)
    ).astype(np.float16)
    bqT = np.ascontiguousarray(bq.reshape(DT_T, 128).T, f32)
    bmT = np.ascontiguousarray(bm.reshape(QT_T, 128).T, f32)
    in_maps = []
    for core in range(N_CORES):
        b, h = core // 2, core % 2
        xl = x[b, h * SL:(h + 1) * SL, :]                      # [1024 s, 1024 q]
        xT = np.ascontiguousarray(
            xl.T.reshape(QT_T, 128, NBLK, SB).transpose(1, 2, 0, 3).reshape(128, -1),
            f32)
        mb = memory[b]                                          # [4096 m, 512 d]
        memT = np.ascontiguousarray(
            mb.reshape(MT_T, 128, DT_T, 128).transpose(3, 0, 2, 1).reshape(128, -1),
            f32)
        memA = np.ascontiguousarray(
            mb.reshape(MT_T, 128, DM).transpose(1, 0, 2).reshape(128, -1), f32)
        in_maps.append({
            "xT": xT, "wqT": wqT, "memT": memT, "memA": memA,
            "wmT": wmT, "bqT": bqT, "bmT": bmT,
        })
    return in_maps


def unpack_output(results, x):
    transformed = np.empty((B, S, DQ), np.float32)
    for core in range(N_CORES):
        b, h = core // 2, core % 2
        o = results[core]["outT"]                               # [128, 8192]
        t_loc = o.reshape(128, NBLK, QT_T, SB).transpose(1, 3, 2, 0).reshape(SL, DQ)
        transformed[b, h * SL:(h + 1) * SL, :] = t_loc
    return transformed


_NC_CACHE = {}


def kernel(x, memory, Wq, bq, Wm, bm):
    x = np.asarray(x, np.float32)
    memory = np.asarray(memory, np.float32)
    Wq = np.asarray(Wq, np.float32)
    bq = np.asarray(bq, np.float32)
    Wm = np.asarray(Wm, np.float32)
    bm = np.asarray(bm, np.float32)
    if "nc" not in _NC_CACHE:
        _NC_CACHE["nc"] = build()
    nc = _NC_CACHE["nc"]
    in_maps = pack_inputs(x, memory, Wq, bq, Wm, bm)
    res = run_bass_kernel_spmd(nc, in_maps, core_ids=list(range(N_CORES)))
    transformed = unpack_output(res.results, x)
    return (x, transformed)



# revision 2
# speedup vs baseline: 4.0058x; 4.0058x over previous
"""Trainium2 Bass kernel for nn_MemoryAdapterLayer (8-core SPMD), pipelined.

reference:
    query = x @ Wq.T + bq                  # [B,S,DM]
    scores = query @ memory.T              # [B,S,M] (per batch)
    weights = softmax(scores, -1)
    attended = weights @ memory            # [B,S,DM]
    transformed = attended @ Wm.T + bm     # [B,S,DQ]
    return (x, transformed)

Sharding: 8 cores = (batch b = core//2) x (sequence half h = core%2).
Each core computes transformed for its [1024, :] slice of one batch.
x is passed through on the host.

On-chip layout is fully transposed ("T" = feature-on-partition):
  step1  QT[d,s]   = sum_q WqT[q,d] * xT[q,s]          (f32r; QT cast bf16)
  step2  sT[m,s]   = sum_d memT[d,m] * QT[d,s]         (bf16)
  exp    eT[m,s]   = exp(sT - SHIFT)                   (ACT, fused shift; bf16)
  step4  aT[d,s]   = sum_m memA[m,d] * eT[m,s]         (bf16)
         Z[s]      = sum_m eT[m,s]                     (f32 DVE acc + ones-matmul)
  norm   attT      = aT * (1/Z) broadcast              (DVE; fp16 out)
  step5  tT[q,s]   = sum_d WmT[d,q] * attT[d,s] + bm   (fp16; fp16 out DMA)

Design notes (all measured on the target cores):
  * The whole j-loop (score + accumulate matmuls, 80% of PE rows) runs
    in ONE uniform 16-bit dtype: on this hardware 16-bit matmuls stream
    ~15% faster than float32r, and mixing weight dtypes between adjacent
    matmul groups costs a further per-switch penalty (~20us/iter when
    alternating fp16/f32r every 4 matmuls). bf16 (not fp16) because the
    exp values exp(s-80) span up to e^65, which needs bf16's fp32-like
    exponent range. bf16 scores cost ~1.3e-2 final rel-err against the
    2e-2 budget: softmax weights only need score accuracy ~1e-1 absolute.
  * Both memory layouts (memT for scores, memA for the weighted sum) are
    SBUF-resident in bf16 (4MB each), so the steady-state HBM traffic is
    just the 2MB fp16 output per iteration -- no streaming DMA to race.
  * The PE executes in issue order, so emission order software-pipelines
    two levels: (1) acc matmuls run two j-iterations behind the score
    matmuls, hiding the ACT exp latency; (2) step1 of the NEXT
    (rep, blk) work and the j0 score/exp preload are emitted inside the
    current work's normalize/step5 tail, so the PE never idles through
    the Z-chain (ones-matmul -> reciprocal -> broadcast -> ATT muls).
  * Z is accumulated in f32 on the DVE from the same bf16 exp tiles the
    matmul consumes, so numerator/denominator quantization cancels.

All DMAs go through SWDGE (gpsimd): this container's walrus rejects
HWDGE semaphore waits on PE instructions. split_overflow_waits() caps
per-instruction sync waits at 1 (S3_LW/CTRL_NO slot limits here).
"""
import sys

import numpy as np
import ml_dtypes

for _p in ("/opt/trn_rl_repo",):
    if _p not in sys.path:
        sys.path.insert(0, _p)

import concourse.bass as bass
import concourse.mybir as mybir
from concourse import tile
from concourse.bass_utils import run_bass_kernel_spmd

B, S, M = 4, 2048, 4096
DQ, DM = 1024, 512
N_CORES = 8
SL = S // 2          # per-core sequence rows
NBLK = 2             # s-blocks of 512 per core
SB = 512             # s-block width (fp32 moving-operand max)
QT_T, DT_T, MT_T = DQ // 128, DM // 128, M // 128  # 8, 4, 32
SHIFT = 80.0

F32R = mybir.dt.float32r
F32 = mybir.dt.float32
F16 = mybir.dt.float16
BF16 = mybir.dt.bfloat16

_counter = [0]


def _split_overflow_waits(nc, limit=1):
    """Walrus here rejects >1 sync wait per instruction: hoist excess waits
    onto same-engine NOPs inserted directly before the instruction."""
    for bb in nc.main_func.blocks:
        insts = list(bb.instructions)
        out = []
        dirty = False
        for ins in insts:
            si = ins.sync_info
            waits = list(si.on_wait) if si is not None else []
            if len(waits) > limit:
                extra = waits[: len(waits) - limit]
                keep = waits[len(waits) - limit:]
                for w in extra:
                    _counter[0] += 1
                    nop = mybir.InstNoOp(
                        name=f"waitfix-{_counter[0]}",
                        engine=ins.engine,
                        sync_info=mybir.SyncInfo(on_wait=[w], on_update=[]),
                        bass_nofuse=True,
                    )
                    nc.register_instruction(nop, overwrite=True)
                    out.append(nop)
                ins.sync_info = mybir.SyncInfo(
                    on_wait=keep, on_update=list(si.on_update)
                )
                dirty = True
            out.append(ins)
        if dirty:
            bb.instructions = out


def build(repeats=1):
    from contextlib import ExitStack

    nc = bass.Bass("TRN2", debug=False, num_devices=N_CORES)
    AF = mybir.ActivationFunctionType

    xT_d = nc.dram_tensor("xT", [128, NBLK * QT_T * SB], F32R, kind="ExternalInput").ap()
    wqT_d = nc.dram_tensor("wqT", [128, QT_T * DT_T * 128], F32R, kind="ExternalInput").ap()
    memT_d = nc.dram_tensor("memT", [128, MT_T * 512], BF16, kind="ExternalInput").ap()
    memA_d = nc.dram_tensor("memA", [128, MT_T * 512], BF16, kind="ExternalInput").ap()
    wmT_d = nc.dram_tensor("wmT", [128, DT_T * QT_T * 128], F16, kind="ExternalInput").ap()
    bqT_d = nc.dram_tensor("bqT", [128, DT_T], F32, kind="ExternalInput").ap()
    bmT_d = nc.dram_tensor("bmT", [128, QT_T], F32, kind="ExternalInput").ap()
    outT_d = nc.dram_tensor("outT", [128, NBLK * QT_T * SB], F16, kind="ExternalOutput").ap()

    with tile.TileContext(nc) as tc:
        with ExitStack() as ctx:
            res = ctx.enter_context(tc.tile_pool(name="res", bufs=1))
            qtp = ctx.enter_context(tc.tile_pool(name="qtp", bufs=8))
            exp = ctx.enter_context(tc.tile_pool(name="expp", bufs=5))
            att = ctx.enter_context(tc.tile_pool(name="attp", bufs=8))
            bcp = ctx.enter_context(tc.tile_pool(name="bcp", bufs=2))
            otp = ctx.enter_context(tc.tile_pool(name="otp", bufs=4))
            ps = ctx.enter_context(tc.tile_pool(name="ps", bufs=3, space="PSUM"))
            psa = ctx.enter_context(tc.tile_pool(name="psa", bufs=1, space="PSUM"))

            # resident tensors
            xT = res.tile([128, NBLK * QT_T * SB], F32R)
            wqT = res.tile([128, QT_T * DT_T * 128], F32R)
            memT = res.tile([128, MT_T * 512], BF16)
            memA = res.tile([128, MT_T * 512], BF16)
            wmT = res.tile([128, DT_T * QT_T * 128], F16)
            bqT = res.tile([128, DT_T], F32)
            bmT = res.tile([128, QT_T], F32)
            ones = res.tile([128, 1], F32)
            onesr = res.tile([1, 128], F32)
            negs = res.tile([128, 1], F32)
            nc.gpsimd.dma_start(xT[:], xT_d)
            nc.gpsimd.dma_start(wqT[:], wqT_d)
            nc.gpsimd.dma_start(memT[:], memT_d)
            nc.gpsimd.dma_start(memA[:], memA_d)
            nc.gpsimd.dma_start(wmT[:], wmT_d)
            nc.gpsimd.dma_start(bqT[:], bqT_d)
            nc.gpsimd.dma_start(bmT[:], bmT_d)
            nc.gpsimd.memset(ones[:], 1.0)
            nc.gpsimd.memset(onesr[:], 1.0)
            nc.gpsimd.memset(negs[:], -SHIFT)

            def emit_s1_group(blk, dt):
                """One step1 psum group: QT[dt] = WqT.T @ xT + bq."""
                pq = ps.tile([128, SB], F32, tag="mm", name="pq")
                for qt in range(QT_T):
                    nc.tensor.matmul(
                        pq[:],
                        wqT[:, (qt * DT_T + dt) * 128:(qt * DT_T + dt + 1) * 128],
                        xT[:, (blk * QT_T + qt) * SB:(blk * QT_T + qt + 1) * SB],
                        start=(qt == 0), stop=(qt == QT_T - 1),
                    )
                q_t = qtp.tile([128, SB], BF16, tag="qt", name="q_t")
                nc.scalar.activation(q_t[:], pq[:], AF.Identity,
                                     bias=bqT[:, dt:dt + 1])
                return q_t

            works = [blk for _rep in range(repeats) for blk in range(NBLK)]

            def emit_ss(QT, j):
                """Score psum group for memory tile j."""
                ss = ps.tile([128, SB], F32, tag="mm", name="ss")
                for dt in range(DT_T):
                    nc.tensor.matmul(
                        ss[:],
                        memT[:, (j * DT_T + dt) * 128:(j * DT_T + dt + 1) * 128],
                        QT[dt][:],
                        start=(dt == 0), stop=(dt == DT_T - 1),
                    )
                return ss

            def emit_exp(ss):
                ex = exp.tile([128, SB], BF16, tag="ex", name="ex")
                nc.scalar.activation(ex[:], ss[:], AF.Exp, bias=negs[:])
                return ex

            # state for the work currently in the j-loop and its successor;
            # j0's scores+exp are preloaded by the predecessor's tail
            QT_cur = [emit_s1_group(works[0], dt) for dt in range(DT_T)]
            ex0_cur = emit_exp(emit_ss(QT_cur, 0))

            for i, blk in enumerate(works):
                nxt = works[i + 1] if i + 1 < len(works) else None

                # ---- steps 2-4 over memory tiles, acc skewed one j back ----
                accs = [psa.tile([128, SB], F32, tag=f"acc{k}", name=f"acc{k}")
                        for k in range(DT_T)]
                sums = psa.tile([1, SB], F32, tag="sums", name="sums")
                zacc = bcp.tile([128, SB], F32, tag="zacc", name="zacc")
                nc.vector.tensor_copy(zacc[:], ex0_cur[:])
                pend = [(ex0_cur, 0)]  # (ex_tile, j) awaiting acc matmuls

                def emit_acc(ex_t, j):
                    for dt in range(DT_T):
                        nc.tensor.matmul(
                            accs[dt][:],
                            memA[:, j * 512 + dt * 128: j * 512 + (dt + 1) * 128],
                            ex_t[:],
                            start=(j == 0), stop=(j == MT_T - 1),
                        )

                # two-iteration skew between scores and acc so the ACT exp
                # latency never gates the acc matmuls
                for j in range(1, MT_T):
                    ss = emit_ss(QT_cur, j)
                    ex = emit_exp(ss)
                    if len(pend) >= 2:
                        emit_acc(*pend.pop(0))
                    nc.vector.tensor_add(zacc[:], zacc[:], ex[:])
                    pend.append((ex, j))
                for p in pend:
                    emit_acc(*p)

                # ---- Z-chain + next work's step1 fill the PE tail ----
                nc.tensor.matmul(sums[:], ones[:], zacc[:], start=True, stop=True)
                QT_nxt = []
                if nxt is not None:
                    QT_nxt.append(emit_s1_group(nxt, 0))
                rc = bcp.tile([1, SB], F32, tag="rc", name="rc")
                nc.vector.reciprocal(rc[:], sums[:])
                bc_ps = ps.tile([128, SB], F32, tag="mm", name="bc_ps")
                nc.tensor.matmul(bc_ps[:], onesr[:], rc[:], start=True, stop=True)
                bc = bcp.tile([128, SB], F32, tag="bc", name="bc")
                nc.scalar.activation(bc[:], bc_ps[:], AF.Copy)
                if nxt is not None:
                    for dt in range(1, DT_T):
                        QT_nxt.append(emit_s1_group(nxt, dt))

                # ---- normalize: attT = accs * (1/Z) ----
                ATT = []
                for dt in range(DT_T):
                    a_t = att.tile([128, SB], F16, tag="att", name="a_t")
                    nc.vector.tensor_mul(a_t[:], accs[dt][:], bc[:])
                    ATT.append(a_t)

                # ---- step5: out[qt] = WmT.T @ attT + bm ----
                # After the first group, preload the next work's j0
                # scores+exp so its exp only queues behind one ot epilogue
                # on ACT (otherwise the next j-loop's first acc matmuls
                # stall on a deep ACT backlog).
                ex0_nxt = None
                for qt in range(QT_T):
                    p5 = ps.tile([128, SB], F32, tag="mm", name="p5")
                    for dt in range(DT_T):
                        nc.tensor.matmul(
                            p5[:],
                            wmT[:, (dt * QT_T + qt) * 128:(dt * QT_T + qt + 1) * 128],
                            ATT[dt][:],
                            start=(dt == 0), stop=(dt == DT_T - 1),
                        )
                    ot = otp.tile([128, SB], F16, tag="ot", name="ot")
                    nc.scalar.activation(ot[:], p5[:], AF.Identity,
                                         bias=bmT[:, qt:qt + 1])
                    nc.gpsimd.dma_start(
                        outT_d[:, (blk * QT_T + qt) * SB:(blk * QT_T + qt + 1) * SB],
                        ot[:],
                    )
                    if qt == 0 and nxt is not None:
                        ex0_nxt = emit_exp(emit_ss(QT_nxt, 0))
                QT_cur = QT_nxt
                ex0_cur = ex0_nxt
    _split_overflow_waits(nc)
    return nc


def pack_inputs(x, memory, Wq, bq, Wm, bm):
    """Host-side pre-swizzle into SBUF-shaped [128, F] per-core arrays."""
    f32 = np.float32
    wqT = np.ascontiguousarray(
        Wq.reshape(DT_T, 128, QT_T, 128).transpose(3, 2, 0, 1).reshape(128, -1), f32)
    wmT = np.ascontiguousarray(
        Wm.reshape(QT_T, 128, DT_T, 128).transpose(3, 2, 0, 1).reshape(128, -1)
    ).astype(np.float16)
    bqT = np.ascontiguousarray(bq.reshape(DT_T, 128).T, f32)
    bmT = np.ascontiguousarray(bm.reshape(QT_T, 128).T, f32)
    in_maps = []
    for core in range(N_CORES):
        b, h = core // 2, core % 2
        xl = x[b, h * SL:(h + 1) * SL, :]                      # [1024 s, 1024 q]
        xT = np.ascontiguousarray(
            xl.T.reshape(QT_T, 128, NBLK, SB).transpose(1, 2, 0, 3).reshape(128, -1),
            f32)
        mb = memory[b]                                          # [4096 m, 512 d]
        memT = np.ascontiguousarray(
            mb.reshape(MT_T, 128, DT_T, 128).transpose(3, 0, 2, 1).reshape(128, -1)
        ).astype(ml_dtypes.bfloat16)
        memA = np.ascontiguousarray(
            mb.reshape(MT_T, 128, DM).transpose(1, 0, 2).reshape(128, -1)
        ).astype(ml_dtypes.bfloat16)
        in_maps.append({
            "xT": xT, "wqT": wqT, "memT": memT, "memA": memA,
            "wmT": wmT, "bqT": bqT, "bmT": bmT,
        })
    return in_maps


def unpack_output(results, x):
    transformed = np.empty((B, S, DQ), np.float32)
    for core in range(N_CORES):
        b, h = core // 2, core % 2
        o = np.asarray(results[core]["outT"], np.float32)       # [128, 8192]
        t_loc = o.reshape(128, NBLK, QT_T, SB).transpose(1, 3, 2, 0).reshape(SL, DQ)
        transformed[b, h * SL:(h + 1) * SL, :] = t_loc
    return transformed


_NC_CACHE = {}


def kernel(x, memory, Wq, bq, Wm, bm):
    x = np.asarray(x, np.float32)
    memory = np.asarray(memory, np.float32)
    Wq = np.asarray(Wq, np.float32)
    bq = np.asarray(bq, np.float32)
    Wm = np.asarray(Wm, np.float32)
    bm = np.asarray(bm, np.float32)
    if "nc" not in _NC_CACHE:
        _NC_CACHE["nc"] = build()
    nc = _NC_CACHE["nc"]
    in_maps = pack_inputs(x, memory, Wq, bq, Wm, bm)
    res = run_bass_kernel_spmd(nc, in_maps, core_ids=list(range(N_CORES)))
    transformed = unpack_output(res.results, x)
    return (x, transformed)


# revision 3
# speedup vs baseline: 4.1880x; 1.0455x over previous
"""Trainium2 Bass kernel for nn_MemoryAdapterLayer (8-core SPMD), pipelined.

reference:
    query = x @ Wq.T + bq                  # [B,S,DM]
    scores = query @ memory.T              # [B,S,M] (per batch)
    weights = softmax(scores, -1)
    attended = weights @ memory            # [B,S,DM]
    transformed = attended @ Wm.T + bm     # [B,S,DQ]
    return (x, transformed)

Sharding: 8 cores = (batch b = core//2) x (sequence half h = core%2).
Each core computes transformed for its [1024, :] slice of one batch.
x is passed through on the host.

On-chip layout is fully transposed ("T" = feature-on-partition):
  step1  QT[d,s]   = sum_q WqT[q,d] * xT[q,s]          (fp16; QT cast bf16)
  step2  sT[m,s]   = sum_d memT[d,m] * QT[d,s]         (bf16)
  exp    eT[m,s]   = exp(sT - SHIFT)                   (ACT, fused shift; bf16)
  step4  aT[d,s]   = sum_m memA[m,d] * eT[m,s]         (bf16)
         Z[s]      = sum_m eT[m,s]                     (f32 DVE acc; bf16 ones-mm)
  norm   attT      = aT * (1/Z) broadcast              (DVE; fp16 out)
  step5  tT[q,s]   = sum_d WmT[d,q] * attT[d,s] + bm   (fp16; fp16 out DMA)

Design notes (all measured on the target cores):
  * The whole j-loop (score + accumulate matmuls, 80% of PE rows) runs
    in ONE uniform 16-bit dtype: on this hardware 16-bit matmuls stream
    ~15% faster than float32r, and mixing weight dtypes between adjacent
    matmul groups costs a further per-switch penalty (~20us/iter when
    alternating fp16/f32r every 4 matmuls). bf16 (not fp16) because the
    exp values exp(s-80) span up to e^65, which needs bf16's fp32-like
    exponent range. bf16 scores cost ~1.3e-2 final rel-err against the
    2e-2 budget: softmax weights only need score accuracy ~1e-1 absolute.
  * Both memory layouts (memT for scores, memA for the weighted sum) are
    SBUF-resident in bf16 (4MB each), so the steady-state HBM traffic is
    just the 2MB fp16 output per iteration -- no streaming DMA to race.
  * The PE executes in issue order, so emission order software-pipelines
    two levels: (1) acc matmuls run two j-iterations behind the score
    matmuls, hiding the ACT exp latency; (2) step1 of the NEXT
    (rep, blk) work and the j0 score/exp preload are emitted inside the
    current work's normalize/step5 tail, so the PE never idles through
    the Z-chain (ones-matmul -> reciprocal -> broadcast -> ATT muls).
  * Z is accumulated in f32 on the DVE from the same bf16 exp tiles the
    matmul consumes, so numerator/denominator quantization cancels. The
    Z-sum and 1/Z-broadcast matmuls take bf16 operands (bf16 covers
    Z in [e^-2, e^65]; fp16 would overflow): plain f32 operands put the
    PE in its 4-cycles-per-row full-precision mode. step1's inputs are
    fp16 for the same 16-bit-rate reason; its error contribution is
    negligible next to the bf16 QT cast the score matmul needs anyway.

All DMAs go through SWDGE (gpsimd): this container's walrus rejects
HWDGE semaphore waits on PE instructions. split_overflow_waits() caps
per-instruction sync waits at 1 (S3_LW/CTRL_NO slot limits here).
"""
import sys

import numpy as np
import ml_dtypes

for _p in ("/opt/trn_rl_repo",):
    if _p not in sys.path:
        sys.path.insert(0, _p)

import concourse.bass as bass
import concourse.mybir as mybir
from concourse import tile
from concourse.bass_utils import run_bass_kernel_spmd

B, S, M = 4, 2048, 4096
DQ, DM = 1024, 512
N_CORES = 8
SL = S // 2          # per-core sequence rows
NBLK = 2             # s-blocks of 512 per core
SB = 512             # s-block width (fp32 moving-operand max)
QT_T, DT_T, MT_T = DQ // 128, DM // 128, M // 128  # 8, 4, 32
SHIFT = 80.0

F32R = mybir.dt.float32r
F32 = mybir.dt.float32
F16 = mybir.dt.float16
BF16 = mybir.dt.bfloat16

_counter = [0]


def _split_overflow_waits(nc, limit=1):
    """Walrus here rejects >1 sync wait per instruction: hoist excess waits
    onto same-engine NOPs inserted directly before the instruction."""
    for bb in nc.main_func.blocks:
        insts = list(bb.instructions)
        out = []
        dirty = False
        for ins in insts:
            si = ins.sync_info
            waits = list(si.on_wait) if si is not None else []
            if len(waits) > limit:
                extra = waits[: len(waits) - limit]
                keep = waits[len(waits) - limit:]
                for w in extra:
                    _counter[0] += 1
                    nop = mybir.InstNoOp(
                        name=f"waitfix-{_counter[0]}",
                        engine=ins.engine,
                        sync_info=mybir.SyncInfo(on_wait=[w], on_update=[]),
                        bass_nofuse=True,
                    )
                    nc.register_instruction(nop, overwrite=True)
                    out.append(nop)
                ins.sync_info = mybir.SyncInfo(
                    on_wait=keep, on_update=list(si.on_update)
                )
                dirty = True
            out.append(ins)
        if dirty:
            bb.instructions = out


def build(repeats=1):
    from contextlib import ExitStack

    nc = bass.Bass("TRN2", debug=False, num_devices=N_CORES)
    AF = mybir.ActivationFunctionType

    xT_d = nc.dram_tensor("xT", [128, NBLK * QT_T * SB], F16, kind="ExternalInput").ap()
    wqT_d = nc.dram_tensor("wqT", [128, QT_T * DT_T * 128], F16, kind="ExternalInput").ap()
    memT_d = nc.dram_tensor("memT", [128, MT_T * 512], BF16, kind="ExternalInput").ap()
    memA_d = nc.dram_tensor("memA", [128, MT_T * 512], BF16, kind="ExternalInput").ap()
    wmT_d = nc.dram_tensor("wmT", [128, DT_T * QT_T * 128], F16, kind="ExternalInput").ap()
    bqT_d = nc.dram_tensor("bqT", [128, DT_T], F32, kind="ExternalInput").ap()
    bmT_d = nc.dram_tensor("bmT", [128, QT_T], F32, kind="ExternalInput").ap()
    outT_d = nc.dram_tensor("outT", [128, NBLK * QT_T * SB], F16, kind="ExternalOutput").ap()

    with tile.TileContext(nc) as tc:
        with ExitStack() as ctx:
            res = ctx.enter_context(tc.tile_pool(name="res", bufs=1))
            qtp = ctx.enter_context(tc.tile_pool(name="qtp", bufs=8))
            exp = ctx.enter_context(tc.tile_pool(name="expp", bufs=5))
            att = ctx.enter_context(tc.tile_pool(name="attp", bufs=8))
            bcp = ctx.enter_context(tc.tile_pool(name="bcp", bufs=2))
            otp = ctx.enter_context(tc.tile_pool(name="otp", bufs=4))
            ps = ctx.enter_context(tc.tile_pool(name="ps", bufs=3, space="PSUM"))
            psa = ctx.enter_context(tc.tile_pool(name="psa", bufs=1, space="PSUM"))

            # resident tensors
            xT = res.tile([128, NBLK * QT_T * SB], F16)
            wqT = res.tile([128, QT_T * DT_T * 128], F16)
            memT = res.tile([128, MT_T * 512], BF16)
            memA = res.tile([128, MT_T * 512], BF16)
            wmT = res.tile([128, DT_T * QT_T * 128], F16)
            bqT = res.tile([128, DT_T], F32)
            bmT = res.tile([128, QT_T], F32)
            ones = res.tile([128, 1], BF16)
            onesr = res.tile([1, 128], BF16)
            negs = res.tile([128, 1], F32)
            nc.gpsimd.dma_start(xT[:], xT_d)
            nc.gpsimd.dma_start(wqT[:], wqT_d)
            nc.gpsimd.dma_start(memT[:], memT_d)
            nc.gpsimd.dma_start(memA[:], memA_d)
            nc.gpsimd.dma_start(wmT[:], wmT_d)
            nc.gpsimd.dma_start(bqT[:], bqT_d)
            nc.gpsimd.dma_start(bmT[:], bmT_d)
            nc.gpsimd.memset(ones[:], 1.0)
            nc.gpsimd.memset(onesr[:], 1.0)
            nc.gpsimd.memset(negs[:], -SHIFT)

            def emit_s1_group(blk, dt):
                """One step1 psum group: QT[dt] = WqT.T @ xT + bq."""
                pq = ps.tile([128, SB], F32, tag="mm", name="pq")
                for qt in range(QT_T):
                    nc.tensor.matmul(
                        pq[:],
                        wqT[:, (qt * DT_T + dt) * 128:(qt * DT_T + dt + 1) * 128],
                        xT[:, (blk * QT_T + qt) * SB:(blk * QT_T + qt + 1) * SB],
                        start=(qt == 0), stop=(qt == QT_T - 1),
                    )
                q_t = qtp.tile([128, SB], BF16, tag="qt", name="q_t")
                nc.scalar.activation(q_t[:], pq[:], AF.Identity,
                                     bias=bqT[:, dt:dt + 1])
                return q_t

            works = [blk for _rep in range(repeats) for blk in range(NBLK)]

            def emit_ss(QT, j):
                """Score psum group for memory tile j."""
                ss = ps.tile([128, SB], F32, tag="mm", name="ss")
                for dt in range(DT_T):
                    nc.tensor.matmul(
                        ss[:],
                        memT[:, (j * DT_T + dt) * 128:(j * DT_T + dt + 1) * 128],
                        QT[dt][:],
                        start=(dt == 0), stop=(dt == DT_T - 1),
                    )
                return ss

            def emit_exp(ss):
                ex = exp.tile([128, SB], BF16, tag="ex", name="ex")
                nc.scalar.activation(ex[:], ss[:], AF.Exp, bias=negs[:])
                return ex

            # state for the work currently in the j-loop and its successor;
            # j0's scores+exp are preloaded by the predecessor's tail
            QT_cur = [emit_s1_group(works[0], dt) for dt in range(DT_T)]
            ex0_cur = emit_exp(emit_ss(QT_cur, 0))

            for i, blk in enumerate(works):
                nxt = works[i + 1] if i + 1 < len(works) else None

                # ---- steps 2-4 over memory tiles, acc skewed one j back ----
                accs = [psa.tile([128, SB], F32, tag=f"acc{k}", name=f"acc{k}")
                        for k in range(DT_T)]
                sums = psa.tile([1, SB], F32, tag="sums", name="sums")
                zacc = bcp.tile([128, SB], F32, tag="zacc", name="zacc")
                nc.vector.tensor_copy(zacc[:], ex0_cur[:])
                pend = [(ex0_cur, 0)]  # (ex_tile, j) awaiting acc matmuls

                def emit_acc(ex_t, j):
                    for dt in range(DT_T):
                        nc.tensor.matmul(
                            accs[dt][:],
                            memA[:, j * 512 + dt * 128: j * 512 + (dt + 1) * 128],
                            ex_t[:],
                            start=(j == 0), stop=(j == MT_T - 1),
                        )

                # two-iteration skew between scores and acc so the ACT exp
                # latency never gates the acc matmuls
                zbf = bcp.tile([128, SB], BF16, tag="zbf", name="zbf")
                for j in range(1, MT_T):
                    ss = emit_ss(QT_cur, j)
                    ex = emit_exp(ss)
                    if len(pend) >= 2:
                        emit_acc(*pend.pop(0))
                    if j < MT_T - 1:
                        nc.vector.tensor_add(zacc[:], zacc[:], ex[:])
                    else:
                        # final add converts the f32 accumulator to bf16 so
                        # the Z-sum matmul runs at 16-bit rate (f32 operands
                        # put the PE in 4-cycles-per-row mode)
                        with nc.allow_low_precision(
                                reason="Z partial sums to bf16: 0.4%/sqrt(128) "
                                       "error on Z, matmul needs 16-bit rate"):
                            nc.vector.tensor_add(zbf[:], zacc[:], ex[:])
                    pend.append((ex, j))
                for p in pend:
                    emit_acc(*p)

                # ---- Z-chain + next work's step1 fill the PE tail ----
                nc.tensor.matmul(sums[:], ones[:], zbf[:], start=True, stop=True)
                QT_nxt = []
                if nxt is not None:
                    QT_nxt.append(emit_s1_group(nxt, 0))
                rc = bcp.tile([1, SB], BF16, tag="rc", name="rc")
                with nc.allow_low_precision(
                        reason="1/Z in bf16: 0.4% row-scale error, inside budget"):
                    nc.vector.reciprocal(rc[:], sums[:])
                bc_ps = ps.tile([128, SB], F32, tag="mm", name="bc_ps")
                nc.tensor.matmul(bc_ps[:], onesr[:], rc[:], start=True, stop=True)
                bc = bcp.tile([128, SB], F32, tag="bc", name="bc")
                nc.scalar.activation(bc[:], bc_ps[:], AF.Copy)
                if nxt is not None:
                    for dt in range(1, DT_T):
                        QT_nxt.append(emit_s1_group(nxt, dt))

                # ---- normalize: attT = accs * (1/Z) ----
                ATT = []
                for dt in range(DT_T):
                    a_t = att.tile([128, SB], F16, tag="att", name="a_t")
                    nc.vector.tensor_mul(a_t[:], accs[dt][:], bc[:])
                    ATT.append(a_t)

                # ---- step5: out[qt] = WmT.T @ attT + bm ----
                # After the first group, preload the next work's j0
                # scores+exp so its exp only queues behind one ot epilogue
                # on ACT (otherwise the next j-loop's first acc matmuls
                # stall on a deep ACT backlog).
                ex0_nxt = None
                for qt in range(QT_T):
                    p5 = ps.tile([128, SB], F32, tag="mm", name="p5")
                    for dt in range(DT_T):
                        nc.tensor.matmul(
                            p5[:],
                            wmT[:, (dt * QT_T + qt) * 128:(dt * QT_T + qt + 1) * 128],
                            ATT[dt][:],
                            start=(dt == 0), stop=(dt == DT_T - 1),
                        )
                    ot = otp.tile([128, SB], F16, tag="ot", name="ot")
                    nc.scalar.activation(ot[:], p5[:], AF.Identity,
                                         bias=bmT[:, qt:qt + 1])
                    nc.gpsimd.dma_start(
                        outT_d[:, (blk * QT_T + qt) * SB:(blk * QT_T + qt + 1) * SB],
                        ot[:],
                    )
                    if qt == 0 and nxt is not None:
                        ex0_nxt = emit_exp(emit_ss(QT_nxt, 0))
                QT_cur = QT_nxt
                ex0_cur = ex0_nxt
    _split_overflow_waits(nc)
    return nc


def pack_inputs(x, memory, Wq, bq, Wm, bm):
    """Host-side pre-swizzle into SBUF-shaped [128, F] per-core arrays."""
    f32 = np.float32
    wqT = np.ascontiguousarray(
        Wq.reshape(DT_T, 128, QT_T, 128).transpose(3, 2, 0, 1).reshape(128, -1)
    ).astype(np.float16)
    wmT = np.ascontiguousarray(
        Wm.reshape(QT_T, 128, DT_T, 128).transpose(3, 2, 0, 1).reshape(128, -1)
    ).astype(np.float16)
    bqT = np.ascontiguousarray(bq.reshape(DT_T, 128).T, f32)
    bmT = np.ascontiguousarray(bm.reshape(QT_T, 128).T, f32)
    in_maps = []
    for core in range(N_CORES):
        b, h = core // 2, core % 2
        xl = x[b, h * SL:(h + 1) * SL, :]                      # [1024 s, 1024 q]
        xT = np.ascontiguousarray(
            xl.T.reshape(QT_T, 128, NBLK, SB).transpose(1, 2, 0, 3).reshape(128, -1)
        ).astype(np.float16)
        mb = memory[b]                                          # [4096 m, 512 d]
        memT = np.ascontiguousarray(
            mb.reshape(MT_T, 128, DT_T, 128).transpose(3, 0, 2, 1).reshape(128, -1)
        ).astype(ml_dtypes.bfloat16)
        memA = np.ascontiguousarray(
            mb.reshape(MT_T, 128, DM).transpose(1, 0, 2).reshape(128, -1)
        ).astype(ml_dtypes.bfloat16)
        in_maps.append({
            "xT": xT, "wqT": wqT, "memT": memT, "memA": memA,
            "wmT": wmT, "bqT": bqT, "bmT": bmT,
        })
    return in_maps


def unpack_output(results, x):
    transformed = np.empty((B, S, DQ), np.float32)
    for core in range(N_CORES):
        b, h = core // 2, core % 2
        o = np.asarray(results[core]["outT"], np.float32)       # [128, 8192]
        t_loc = o.reshape(128, NBLK, QT_T, SB).transpose(1, 3, 2, 0).reshape(SL, DQ)
        transformed[b, h * SL:(h + 1) * SL, :] = t_loc
    return transformed


_NC_CACHE = {}


def kernel(x, memory, Wq, bq, Wm, bm):
    x = np.asarray(x, np.float32)
    memory = np.asarray(memory, np.float32)
    Wq = np.asarray(Wq, np.float32)
    bq = np.asarray(bq, np.float32)
    Wm = np.asarray(Wm, np.float32)
    bm = np.asarray(bm, np.float32)
    if "nc" not in _NC_CACHE:
        _NC_CACHE["nc"] = build()
    nc = _NC_CACHE["nc"]
    in_maps = pack_inputs(x, memory, Wq, bq, Wm, bm)
    res = run_bass_kernel_spmd(nc, in_maps, core_ids=list(range(N_CORES)))
    transformed = unpack_output(res.results, x)
    return (x, transformed)
